# revision 41
# baseline (speedup 1.0000x reference)
"""Trainium2 Bass kernel for AttentiveMinkUNetDiff KNN+MLP block (v2).

Self-contained: hardcodes shapes N=16384, M=32768, K=8, C=256, 8 cores.
Sharding: nodes across 8 cores; cond set replicated.

Per core (2048 nodes, 16 tiles of 128):
  1. PE: exact bf16-split integer matmul (K=19 rows) producing a
     per-node-ranking-equivalent of -40000*d^2 for all 32768 cond points.
  2. ACT copies PSUM->SBUF row buffers; DVE max/max_index per 8192-wide
     super -> 32-candidate pool with within-super indices.
  3. Top-16 of pool by value (match_replace rounds), then re-sorted by
     ascending cond index (jax.lax.top_k tie order) via masked-max trick.
  4. One packed indirect-DMA gather per candidate ([coords|feats] rows);
     exact d^2 recomputed bit-exactly vs XLA's fused fma chain (Dekker).
  5. Final 8 by exact value; inverse-distance weights; weighted mean of
     feats via ACT scaling + PE transpose-accumulate (weights sum to 1 so
     the mean commutes with W_proj); 3-layer MLP in transposed space;
     timestep-embedding branch folded into the final bias.
"""
import math
import numpy as np
import ml_dtypes

import concourse.bass as bass
import concourse.mybir as mybir
from concourse.tile import TileContext
from concourse import bass_utils
from concourse import bacc

bf16 = ml_dtypes.bfloat16
f32 = np.float32
AF = mybir.ActivationFunctionType
OP = mybir.AluOpType

N, M, K = 16384, 32768, 8
C = 256
PACK = C + 4                  # packed row: [part_c(4) | feats(256)]
EMBED, HALF = 96, 48
NCORES = 8
NSHARD = N // NCORES          # 2048
NTILES = NSHARD // 128        # 16
SUP = 8192                    # super-chunk width scanned from SBUF
NSUP = M // SUP               # 4
POOL = NSUP * 8               # 32
NCAND = 16
PI = float(np.pi)


# ---------------------------------------------------------------- host prep
def _split_rows(nodes, conds):
    """Build the 19 bf16-exact contraction rows. Validated vs reference."""
    a = nodes[:, 1:4].astype(np.int64)
    b = conds[:, 1:4].astype(np.int64)
    ah, al = a >> 5, a & 31
    bh, bl = b >> 5, b & 31
    lhs, rhs = [], []
    for k in range(3):
        lhs += [1280.0 * ah[:, k], 1280.0 * ah[:, k], 40.0 * al[:, k], 40.0 * al[:, k]]
        rhs += [32.0 * bh[:, k], 1.0 * bl[:, k], 32.0 * bh[:, k], 1.0 * bl[:, k]]
    B_total = (4 * b * b - 316 * b).sum(1) + 32768
    s2, s1, s0 = B_total >> 16, (B_total >> 8) & 255, B_total & 255
    nones = -np.ones(a.shape[0])
    lhs += [nones, nones, nones]
    rhs += [s2 * 65536.0, s1 * 256.0, s0 * 1.0]
    C_i = ((10 * a + 79) ** 2).sum(1) - 32768
    c3 = np.floor(C_i / 2 ** 21).astype(np.int64)
    r = C_i - c3 * 2 ** 21
    c2, c1, c0 = r >> 13, (r >> 5) & 255, r & 31
    mones = np.ones(b.shape[0])
    lhs += [-c3 * 2097152.0, -c2 * 8192.0, -c1 * 32.0, -c0 * 1.0]
    rhs += [mones, mones, mones, mones]
    LHS = np.stack(lhs).astype(f32)   # [19, N]
    RHS = np.stack(rhs).astype(f32)   # [19, M]
    return LHS.astype(bf16), RHS.astype(bf16)


def _transform(coords, stride, voxel, mc):
    c = coords.astype(np.float32)
    batch = (c[:, :1] * f32(mc * f32(2.0))).astype(f32)
    xyz = ((c[:, 1:] + f32(stride / 2.0)).astype(f32) * f32(voxel)).astype(f32)
    return np.concatenate([batch, xyz], 1).astype(f32)


def _pack_w(w):
    """W [dout, din] -> lhsT pack [128, 4*128]: col block (ct*2+dt)."""
    wt = np.ascontiguousarray(w.T.astype(f32))          # [din, dout]
    p = wt.reshape(2, 128, 2, 128)                      # [ct, c, dt, d]
    p = p.transpose(1, 0, 2, 3).reshape(128, 512)
    return np.ascontiguousarray(p)


_CACHE = {}


def _build_program():
    if 'nc' in _CACHE:
        return _CACHE['nc']
    nc = bacc.Bacc("TRN2", target_bir_lowering=False, debug=False,
                   num_devices=NCORES)
    dt = mybir.dt

    def din(name, shape, dtype):
        return nc.dram_tensor(name, shape, dtype, kind="ExternalInput").ap()

    lhsT = din('lhsT', [19, NSHARD], dt.bfloat16)
    rhs = din('rhsT', [19, M], dt.bfloat16)
    nodex = din('nodex', [128, NTILES * 3], dt.float32)
    packed = din('packed', [M, PACK], dt.float32)
    invbase = din('invbase', [128, POOL], dt.float32)
    eye = din('eye', [128, 128], dt.float32)
    wp = din('wp', [128, 512], dt.float32)
    wl1 = din('wl1', [128, 512], dt.float32)
    wl2 = din('wl2', [128, 512], dt.float32)
    bproj = din('bproj', [128, 2], dt.float32)
    bl1 = din('bl1', [128, 2], dt.float32)
    bcomb = din('bcomb', [128, 2], dt.float32)
    wt1 = din('wt1', [EMBED, EMBED], dt.float32)
    wt2 = din('wt2', [EMBED, C], dt.float32)
    bt1 = din('bt1', [EMBED, 1], dt.float32)
    freqs = din('freqs', [EMBED, 1], dt.float32)
    shifts = din('shifts', [EMBED, 1], dt.float32)
    tval = din('tval', [EMBED, 1], dt.float32)
    out = nc.dram_tensor('out', [NSHARD, C], dt.float32, kind="ExternalOutput").ap()

    with TileContext(nc) as tc, \
            tc.tile_pool(name="const", bufs=1) as cpool, \
            tc.tile_pool(name="work", bufs=2) as wpool, \
            tc.tile_pool(name="psum", bufs=2, space="PSUM") as ppool:

        # ---- constants to SBUF
        rhs_sb = cpool.tile([19, M], dt.bfloat16, tag="rhs")
        nc.sync.dma_start(out=rhs_sb[:], in_=rhs)
        lhs_sb = cpool.tile([19, NSHARD], dt.bfloat16, tag="lhs")
        nc.sync.dma_start(out=lhs_sb[:], in_=lhsT)
        nodex_sb = cpool.tile([128, NTILES * 3], dt.float32, tag="nodex")
        nc.sync.dma_start(out=nodex_sb[:], in_=nodex)
        invb_sb = cpool.tile([128, POOL], dt.float32, tag="invb")
        nc.sync.dma_start(out=invb_sb[:], in_=invbase)
        eye_sb = cpool.tile([128, 128], dt.float32, tag="eye")
        nc.sync.dma_start(out=eye_sb[:], in_=eye)
        wp_sb = cpool.tile([128, 512], dt.float32, tag="wp")
        nc.sync.dma_start(out=wp_sb[:], in_=wp)
        wl1_sb = cpool.tile([128, 512], dt.float32, tag="wl1")
        nc.sync.dma_start(out=wl1_sb[:], in_=wl1)
        wl2_sb = cpool.tile([128, 512], dt.float32, tag="wl2")
        nc.sync.dma_start(out=wl2_sb[:], in_=wl2)
        bproj_sb = cpool.tile([128, 2], dt.float32, tag="bproj")
        nc.sync.dma_start(out=bproj_sb[:], in_=bproj)
        bl1_sb = cpool.tile([128, 2], dt.float32, tag="bl1")
        nc.sync.dma_start(out=bl1_sb[:], in_=bl1)
        bcomb_sb = cpool.tile([128, 2], dt.float32, tag="bcomb")
        nc.sync.dma_start(out=bcomb_sb[:], in_=bcomb)
        wt1_sb = cpool.tile([EMBED, EMBED], dt.float32, tag="wt1")
        nc.sync.dma_start(out=wt1_sb[:], in_=wt1)
        wt2_sb = cpool.tile([EMBED, C], dt.float32, tag="wt2")
        nc.sync.dma_start(out=wt2_sb[:], in_=wt2)
        bt1_sb = cpool.tile([EMBED, 1], dt.float32, tag="bt1")
        nc.sync.dma_start(out=bt1_sb[:], in_=bt1)
        fr_sb = cpool.tile([EMBED, 1], dt.float32, tag="fr")
        nc.sync.dma_start(out=fr_sb[:], in_=freqs)
        sh_sb = cpool.tile([EMBED, 1], dt.float32, tag="sh")
        nc.sync.dma_start(out=sh_sb[:], in_=shifts)
        t_sb = cpool.tile([EMBED, 1], dt.float32, tag="t1x1")
        nc.sync.dma_start(out=t_sb[:], in_=tval)

        # ---- t branch -> fincol [128, 2]
        e = cpool.tile([EMBED, 1], dt.float32, tag="e")
        nc.vector.tensor_mul(e[:], t_sb[:], fr_sb[:])
        nc.vector.tensor_add(e[:], e[:], sh_sb[:])
        ki = cpool.tile([EMBED, 1], dt.int32, tag="ki")
        kf = cpool.tile([EMBED, 1], dt.float32, tag="kf")
        nc.vector.tensor_scalar(kf[:], e[:], float(1.0 / (2 * PI)), None, op0=OP.mult)
        nc.vector.tensor_copy(out=ki[:], in_=kf[:])
        nc.vector.tensor_copy(out=kf[:], in_=ki[:])
        nc.vector.tensor_scalar(kf[:], kf[:], float(2 * PI), None, op0=OP.mult)
        nc.vector.tensor_sub(e[:], e[:], kf[:])
        gt = cpool.tile([EMBED, 1], dt.float32, tag="gt")
        nc.vector.tensor_scalar(gt[:], e[:], float(PI), None, op0=OP.is_gt)
        nc.vector.tensor_scalar(gt[:], gt[:], float(2 * PI), None, op0=OP.mult)
        nc.vector.tensor_sub(e[:], e[:], gt[:])
        emb_sb = cpool.tile([EMBED, 1], dt.float32, tag="emb")
        nc.scalar.activation(emb_sb[:], e[:], AF.Sin)
        ps_t1 = ppool.tile([EMBED, 1], dt.float32, tag="mm")
        nc.tensor.matmul(ps_t1[:], lhsT=wt1_sb[:], rhs=emb_sb[:], start=True, stop=True)
        h96 = cpool.tile([EMBED, 1], dt.float32, tag="h96")
        nc.scalar.activation(h96[:], ps_t1[:], AF.Identity, bias=bt1_sb[:, 0:1])
        h96b = cpool.tile([EMBED, 1], dt.float32, tag="h96b")
        nc.vector.tensor_scalar(h96b[:], h96[:], 0.1, None, op0=OP.mult)
        nc.vector.tensor_max(h96b[:], h96b[:], h96[:])
        fincol = cpool.tile([128, 2], dt.float32, tag="fincol")
        for d in range(2):
            ps_t2 = ppool.tile([128, 1], dt.float32, tag="mm")
            nc.tensor.matmul(ps_t2[:], lhsT=wt2_sb[:, d * 128:(d + 1) * 128],
                             rhs=h96b[:], start=True, stop=True)
            nc.scalar.activation(fincol[:, d:d + 1], ps_t2[:], AF.Identity,
                                 bias=bcomb_sb[:, d:d + 1])

        c32k = cpool.tile([128, NCAND], dt.float32, tag="c32k")
        nc.vector.memset(c32k[:], 32768.0)

        # ---- main loop, software-pipelined: A(t+1) emitted before B(t)
        def stageA(t):
            st = {}
            lt = lhs_sb[:, t * 128:(t + 1) * 128]
            pool_vals = wpool.tile([128, POOL], dt.float32, tag="pvals", name="pvals")
            pool_lidx = wpool.tile([128, POOL], dt.uint16, tag="plidx", name="plidx")
            for s in range(NSUP):
                rowb = wpool.tile([128, SUP], dt.float32, tag="rowb", name="rowb")
                for h in range(SUP // 1024):
                    ps_d = ppool.tile([128, 1024], dt.float32, tag="dist", name="psd")
                    for q in range(2):
                        c0 = s * SUP + h * 1024 + q * 512
                        nc.tensor.matmul(ps_d[:, q * 512:(q + 1) * 512], lhsT=lt,
                                         rhs=rhs_sb[:, c0:c0 + 512],
                                         start=True, stop=True)
                    nc.scalar.activation(rowb[:, h * 1024:(h + 1) * 1024], ps_d[:],
                                         AF.Identity)
                nc.vector.max(out=pool_vals[:, s * 8:(s + 1) * 8], in_=rowb[:])
                nc.vector.max_index(out=pool_lidx[:, s * 8:(s + 1) * 8],
                                    in_max=pool_vals[:, s * 8:(s + 1) * 8],
                                    in_values=rowb[:])
            lidxf = wpool.tile([128, POOL], dt.float32, tag="lidxf", name="lidxf")
            nc.vector.tensor_copy(out=lidxf[:], in_=pool_lidx[:])
            pinv = wpool.tile([128, POOL], dt.float32, tag="pinv", name="pinv")
            nc.vector.tensor_sub(pinv[:], invb_sb[:], lidxf[:])
            pv2 = wpool.tile([128, POOL], dt.float32, tag="pv2", name="pv2")
            pv3 = wpool.tile([128, POOL], dt.float32, tag="pv3", name="pv3")
            v8 = wpool.tile([128, 8], dt.float32, tag="v8", name="v8")
            nc.vector.max(out=v8[:], in_=pool_vals[:])
            nc.vector.match_replace(out=pv2[:], in_to_replace=v8[:],
                                    in_values=pool_vals[:], imm_value=-3e38)
            nc.vector.max(out=v8[:], in_=pv2[:])
            nc.vector.match_replace(out=pv3[:], in_to_replace=v8[:],
                                    in_values=pv2[:], imm_value=-3e38)
            maskp = wpool.tile([128, POOL], dt.float32, tag="maskp", name="maskp")
            nc.vector.tensor_tensor(out=maskp[:], in0=pv3[:], in1=pool_vals[:],
                                    op=OP.not_equal)
            nc.vector.tensor_mul(maskp[:], maskp[:], pinv[:])
            inv16 = wpool.tile([128, NCAND], dt.float32, tag="inv16", name="inv16")
            mv2 = wpool.tile([128, POOL], dt.float32, tag="mv2", name="mv2")
            nc.vector.max(out=inv16[:, 0:8], in_=maskp[:])
            nc.vector.match_replace(out=mv2[:], in_to_replace=inv16[:, 0:8],
                                    in_values=maskp[:], imm_value=0.0)
            nc.vector.max(out=inv16[:, 8:16], in_=mv2[:])
            g16f = wpool.tile([128, NCAND], dt.float32, tag="g16f", name="g16f")
            nc.vector.tensor_sub(g16f[:], c32k[:], inv16[:])
            g16u = wpool.tile([128, NCAND], dt.uint32, tag="g16u", name="g16u")
            nc.vector.tensor_copy(out=g16u[:], in_=g16f[:])
            gp = wpool.tile([128, NCAND, PACK], dt.float32, tag="gp", name="gp")
            for k in range(NCAND):
                nc.gpsimd.indirect_dma_start(
                    out=gp[:, k, :], out_offset=None, in_=packed,
                    in_offset=bass.IndirectOffsetOnAxis(ap=g16u[:, k:k + 1], axis=0))
            st['gp'] = gp
            return st

        def stageB(t, st):
            gp = st['gp']
            xs = [nodex_sb[:, t * 3 + k: t * 3 + k + 1] for k in range(3)]
            dcol = [wpool.tile([128, NCAND], dt.float32, tag=f"d{k}",
                               name=f"dcol{k}") for k in range(3)]
            for k in range(3):
                nc.vector.tensor_scalar(dcol[k][:], gp[:, :, k + 1], xs[k],
                                        None, op0=OP.subtract)
            acc = wpool.tile([128, NCAND], dt.float32, tag="acc", name="acc")
            nc.vector.tensor_mul(acc[:], dcol[0][:], dcol[0][:])
            tt = [wpool.tile([128, NCAND], dt.float32, tag=f"t{i}",
                             name=f"tt{i}") for i in range(6)]
            for k in (1, 2):
                d = dcol[k]
                T0, T1, T2, T3, T4, T5 = tt
                nc.vector.tensor_scalar(T0[:], d[:], 4097.0, None, op0=OP.mult)
                nc.vector.tensor_sub(T1[:], T0[:], d[:])
                nc.vector.tensor_sub(T0[:], T0[:], T1[:])
                nc.vector.tensor_sub(T1[:], d[:], T0[:])
                nc.vector.tensor_mul(T2[:], d[:], d[:])
                nc.vector.tensor_mul(T3[:], T0[:], T0[:])
                nc.vector.tensor_sub(T3[:], T3[:], T2[:])
                nc.vector.tensor_add(T4[:], T1[:], T1[:])
                nc.vector.tensor_mul(T4[:], T0[:], T4[:])
                nc.vector.tensor_add(T3[:], T3[:], T4[:])
                nc.vector.tensor_mul(T4[:], T1[:], T1[:])
                nc.vector.tensor_add(T3[:], T3[:], T4[:])
                nc.vector.tensor_add(T4[:], T2[:], acc[:])
                nc.vector.tensor_sub(T5[:], T4[:], T2[:])
                nc.vector.tensor_sub(T0[:], T4[:], T5[:])
                nc.vector.tensor_sub(T0[:], T2[:], T0[:])
                nc.vector.tensor_sub(T1[:], acc[:], T5[:])
                nc.vector.tensor_add(T0[:], T0[:], T1[:])
                nc.vector.tensor_add(T0[:], T0[:], T3[:])
                nc.vector.tensor_add(acc[:], T4[:], T0[:])
            nd2 = wpool.tile([128, NCAND], dt.float32, tag="nd2", name="nd2")
            nc.vector.tensor_scalar(nd2[:], acc[:], -1.0, None, op0=OP.mult)
            v8f = wpool.tile([128, 8], dt.float32, tag="v8f", name="v8f")
            nc.vector.max(out=v8f[:], in_=nd2[:])
            refb = wpool.tile([128, NCAND], dt.float32, tag="refb", name="refb")
            nc.vector.match_replace(out=refb[:], in_to_replace=v8f[:],
                                    in_values=nd2[:], imm_value=3e38)
            mask2 = wpool.tile([128, NCAND], dt.float32, tag="mask2", name="mask2")
            nc.vector.tensor_tensor(out=mask2[:], in0=refb[:], in1=nd2[:],
                                    op=OP.not_equal)
            dist = wpool.tile([128, NCAND], dt.float32, tag="dist16", name="dist16")
            nc.scalar.activation(dist[:], acc[:], AF.Sqrt)
            nc.vector.tensor_scalar_max(dist[:], dist[:], 1e-6)
            wr = wpool.tile([128, NCAND], dt.float32, tag="wr", name="wr")
            nc.vector.reciprocal(out=wr[:], in_=dist[:])
            nc.vector.tensor_mul(wr[:], wr[:], mask2[:])
            wsum = wpool.tile([128, 1], dt.float32, tag="wsum", name="wsum")
            nc.vector.tensor_reduce(out=wsum[:], in_=wr[:],
                                    axis=mybir.AxisListType.X, op=OP.add)
            wsr = wpool.tile([128, 1], dt.float32, tag="wsr", name="wsr")
            nc.vector.reciprocal(out=wsr[:], in_=wsum[:])
            wn = wpool.tile([128, NCAND], dt.float32, tag="wn", name="wn")
            nc.vector.tensor_scalar(wn[:], wr[:], wsr[:, 0:1], None, op0=OP.mult)

            ps_fT = ppool.tile([128, C], dt.float32, tag="tr", name="psft")
            for k in range(NCAND):
                sc = wpool.tile([128, C], dt.float32, tag="sc", name="sc", bufs=3)
                nc.scalar.activation(sc[:], gp[:, k, 4:4 + C], AF.Identity,
                                     scale=wn[:, k:k + 1])
                for half_i in range(2):
                    nc.tensor.matmul(
                        ps_fT[:, half_i * 128:(half_i + 1) * 128],
                        lhsT=sc[:, half_i * 128:(half_i + 1) * 128],
                        rhs=eye_sb[:], is_transpose=True,
                        start=(k == 0 and half_i == 0),
                        stop=(k == NCAND - 1 and half_i == 1))
            fT = wpool.tile([128, C], dt.float32, tag="fT", name="fT")
            nc.scalar.activation(fT[:], ps_fT[:], AF.Identity)

            def dense(src_sb, wpack, bias_sb, leaky, outtag):
                o = wpool.tile([128, C], dt.float32, tag=outtag, name=outtag)
                for d in range(2):
                    ps = ppool.tile([128, 128], dt.float32, tag="mm", name="psmm")
                    for ct in range(2):
                        nc.tensor.matmul(
                            ps[:], lhsT=wpack[:, (ct * 2 + d) * 128:(ct * 2 + d + 1) * 128],
                            rhs=src_sb[:, ct * 128:(ct + 1) * 128],
                            start=(ct == 0), stop=(ct == 1))
                    nc.scalar.activation(o[:, d * 128:(d + 1) * 128], ps[:],
                                         AF.Identity, bias=bias_sb[:, d:d + 1])
                if leaky:
                    tmp = wpool.tile([128, C], dt.float32, tag=outtag + "lk",
                                     name=outtag + "lk")
                    nc.vector.tensor_scalar(tmp[:], o[:], 0.1, None, op0=OP.mult)
                    nc.vector.tensor_max(o[:], o[:], tmp[:])
                return o

            mT = dense(fT, wp_sb, bproj_sb, False, "mT")
            h1T = dense(mT, wl1_sb, bl1_sb, True, "h1T")
            pT = dense(h1T, wl2_sb, fincol, False, "pT")
            osb = wpool.tile([128, C], dt.float32, tag="osb", name="osb")
            for dth in range(2):
                ps_tr = ppool.tile([128, 128], dt.float32, tag="mm", name="pstr")
                nc.tensor.matmul(ps_tr[:], lhsT=pT[:, dth * 128:(dth + 1) * 128],
                                 rhs=eye_sb[:], is_transpose=True,
                                 start=True, stop=True)
                nc.scalar.activation(osb[:, dth * 128:(dth + 1) * 128], ps_tr[:],
                                     AF.Identity)
            nc.sync.dma_start(out=out[t * 128:(t + 1) * 128, :], in_=osb[:])

        pending = None
        for t in range(NTILES + 1):
            if t < NTILES:
                st = stageA(t)
            if pending is not None:
                stageB(t - 1, pending)
            pending = st if t < NTILES else None

    nc.compile()
    _CACHE['nc'] = nc
    return nc


# ---------------------------------------------------------------- host entry
def kernel(node_coords, cond_coords, cond_feats, t,
           W_proj, b_proj, W_l1, b_l1, W_l2, b_l2, W_t1, b_t1, W_t2, b_t2):
    node_coords = np.asarray(node_coords)
    cond_coords = np.asarray(cond_coords)
    cond_feats = np.asarray(cond_feats, dtype=np.float32)
    mc = np.float32(node_coords.astype(np.float32).max())
    part_c = _transform(cond_coords, 1.0, 0.01, mc)
    packed = np.ascontiguousarray(np.concatenate([part_c, cond_feats], 1))
    LHS, RHS = _split_rows(node_coords, cond_coords)
    full_c = _transform(node_coords, 16.0, 0.05, mc)

    invbase = np.tile((32768.0 - (np.arange(POOL) // 8) * SUP).astype(f32)[None, :],
                      (128, 1))
    eye = np.eye(128, dtype=f32)
    freqs1 = np.exp(np.arange(HALF, dtype=np.float32) *
                    f32(-math.log(10000.0) / (HALF - 1))).astype(f32)
    freqs = np.concatenate([freqs1, freqs1])
    shifts = np.concatenate([np.zeros(HALF, f32), np.full(HALF, PI / 2, f32)])

    nc = _build_program()
    in_maps = []
    for i in range(NCORES):
        sl = slice(i * NSHARD, (i + 1) * NSHARD)
        nodex = np.ascontiguousarray(
            full_c[sl, 1:4].reshape(NTILES, 128, 3).transpose(1, 0, 2)
            .reshape(128, NTILES * 3))
        in_maps.append({
            'lhsT': np.ascontiguousarray(LHS[:, sl]),
            'rhsT': RHS,
            'nodex': nodex,
            'packed': packed,
            'invbase': invbase,
            'eye': eye,
            'wp': _pack_w(np.asarray(W_proj, dtype=f32)),
            'wl1': _pack_w(np.asarray(W_l1, dtype=f32)),
            'wl2': _pack_w(np.asarray(W_l2, dtype=f32)),
            'bproj': np.asarray(b_proj, f32).reshape(2, 128).T.copy(),
            'bl1': np.asarray(b_l1, f32).reshape(2, 128).T.copy(),
            'bcomb': (np.asarray(b_l2, f32) + np.asarray(b_t2, f32)).reshape(2, 128).T.copy(),
            'wt1': np.ascontiguousarray(np.asarray(W_t1, f32).T),
            'wt2': np.ascontiguousarray(np.asarray(W_t2, f32).T),
            'bt1': np.asarray(b_t1, f32).reshape(EMBED, 1).copy(),
            'freqs': freqs.reshape(EMBED, 1).copy(),
            'shifts': shifts.reshape(EMBED, 1).copy(),
            'tval': np.full((EMBED, 1), np.asarray(t, f32).reshape(()), f32),
        })
    res = bass_utils.run_bass_kernel_spmd(nc, in_maps, core_ids=list(range(NCORES)))
    _CACHE['last_result'] = res
    outs = [res.results[i]['out'] for i in range(NCORES)]
    return np.concatenate(outs, 0)


# revision 44
# speedup vs baseline: 1.0277x; 1.0277x over previous
"""Trainium2 Bass kernel for AttentiveMinkUNetDiff KNN+MLP block (v2).

Self-contained: hardcodes shapes N=16384, M=32768, K=8, C=256, 8 cores.
Sharding: nodes across 8 cores; cond set replicated.

Per core (2048 nodes, 16 tiles of 128):
  1. PE: exact bf16-split integer matmul (K=19 rows) producing a
     per-node-ranking-equivalent of -40000*d^2 for all 32768 cond points.
  2. ACT copies PSUM->SBUF row buffers; DVE max/max_index per 8192-wide
     super -> 32-candidate pool with within-super indices.
  3. Top-16 of pool by value (match_replace rounds), then re-sorted by
     ascending cond index (jax.lax.top_k tie order) via masked-max trick.
  4. One packed indirect-DMA gather per candidate ([coords|feats] rows);
     exact d^2 recomputed bit-exactly vs XLA's fused fma chain (Dekker).
  5. Final 8 by exact value; inverse-distance weights; weighted mean of
     feats via ACT scaling + PE transpose-accumulate (weights sum to 1 so
     the mean commutes with W_proj); 3-layer MLP in transposed space;
     timestep-embedding branch folded into the final bias.
"""
import math
import numpy as np
import ml_dtypes

import concourse.bass as bass
import concourse.mybir as mybir
from concourse.tile import TileContext
from concourse import bass_utils
from concourse import bacc

bf16 = ml_dtypes.bfloat16
f32 = np.float32
AF = mybir.ActivationFunctionType
OP = mybir.AluOpType

N, M, K = 16384, 32768, 8
C = 256
PACK = C + 4                  # packed row: [part_c(4) | feats(256)]
EMBED, HALF = 96, 48
NCORES = 8
NSHARD = N // NCORES          # 2048
NTILES = NSHARD // 128        # 16
SUP = 4096                    # super-chunk width scanned from SBUF
NSUP = M // SUP               # 8
POOL = NSUP * 8               # 64
NCAND = 16
PI = float(np.pi)


# ---------------------------------------------------------------- host prep
def _split_rows(nodes, conds):
    """Build the 19 bf16-exact contraction rows. Validated vs reference."""
    a = nodes[:, 1:4].astype(np.int64)
    b = conds[:, 1:4].astype(np.int64)
    ah, al = a >> 5, a & 31
    bh, bl = b >> 5, b & 31
    lhs, rhs = [], []
    for k in range(3):
        lhs += [1280.0 * ah[:, k], 1280.0 * ah[:, k], 40.0 * al[:, k], 40.0 * al[:, k]]
        rhs += [32.0 * bh[:, k], 1.0 * bl[:, k], 32.0 * bh[:, k], 1.0 * bl[:, k]]
    B_total = (4 * b * b - 316 * b).sum(1) + 32768
    s2, s1, s0 = B_total >> 16, (B_total >> 8) & 255, B_total & 255
    nones = -np.ones(a.shape[0])
    lhs += [nones, nones, nones]
    rhs += [s2 * 65536.0, s1 * 256.0, s0 * 1.0]
    C_i = ((10 * a + 79) ** 2).sum(1) - 32768
    c3 = np.floor(C_i / 2 ** 21).astype(np.int64)
    r = C_i - c3 * 2 ** 21
    c2, c1, c0 = r >> 13, (r >> 5) & 255, r & 31
    mones = np.ones(b.shape[0])
    lhs += [-c3 * 2097152.0, -c2 * 8192.0, -c1 * 32.0, -c0 * 1.0]
    rhs += [mones, mones, mones, mones]
    LHS = np.stack(lhs).astype(f32)   # [19, N]
    RHS = np.stack(rhs).astype(f32)   # [19, M]
    return LHS.astype(bf16), RHS.astype(bf16)


def _transform(coords, stride, voxel, mc):
    c = coords.astype(np.float32)
    batch = (c[:, :1] * f32(mc * f32(2.0))).astype(f32)
    xyz = ((c[:, 1:] + f32(stride / 2.0)).astype(f32) * f32(voxel)).astype(f32)
    return np.concatenate([batch, xyz], 1).astype(f32)


def _pack_w(w):
    """W [dout, din] -> lhsT pack [128, 4*128]: col block (ct*2+dt)."""
    wt = np.ascontiguousarray(w.T.astype(f32))          # [din, dout]
    p = wt.reshape(2, 128, 2, 128)                      # [ct, c, dt, d]
    p = p.transpose(1, 0, 2, 3).reshape(128, 512)
    return np.ascontiguousarray(p)


_CACHE = {}


def _build_program():
    if 'nc' in _CACHE:
        return _CACHE['nc']
    nc = bacc.Bacc("TRN2", target_bir_lowering=False, debug=False,
                   num_devices=NCORES)
    dt = mybir.dt

    def din(name, shape, dtype):
        return nc.dram_tensor(name, shape, dtype, kind="ExternalInput").ap()

    lhsT = din('lhsT', [19, NSHARD], dt.bfloat16)
    rhs = din('rhsT', [19, M], dt.bfloat16)
    nodex = din('nodex', [128, NTILES * 3], dt.float32)
    packed = din('packed', [M, PACK], dt.float32)
    invbase = din('invbase', [128, POOL], dt.float32)
    eye = din('eye', [128, 128], dt.float32)
    wp = din('wp', [128, 512], dt.float32)
    wl1 = din('wl1', [128, 512], dt.float32)
    wl2 = din('wl2', [128, 512], dt.float32)
    bproj = din('bproj', [128, 2], dt.float32)
    bl1 = din('bl1', [128, 2], dt.float32)
    bcomb = din('bcomb', [128, 2], dt.float32)
    wt1 = din('wt1', [EMBED, EMBED], dt.float32)
    wt2 = din('wt2', [EMBED, C], dt.float32)
    bt1 = din('bt1', [EMBED, 1], dt.float32)
    freqs = din('freqs', [EMBED, 1], dt.float32)
    shifts = din('shifts', [EMBED, 1], dt.float32)
    tval = din('tval', [EMBED, 1], dt.float32)
    out = nc.dram_tensor('out', [NSHARD, C], dt.float32, kind="ExternalOutput").ap()

    with TileContext(nc) as tc, \
            tc.tile_pool(name="const", bufs=1) as cpool, \
            tc.tile_pool(name="work", bufs=2) as wpool, \
            tc.tile_pool(name="psum", bufs=2, space="PSUM") as ppool:

        # ---- constants to SBUF
        rhs_sb = cpool.tile([19, M], dt.bfloat16, tag="rhs")
        for j in range(8):
            nc.sync.dma_start(out=rhs_sb[:, j * (M // 8):(j + 1) * (M // 8)],
                              in_=rhs[:, j * (M // 8):(j + 1) * (M // 8)])
        lhs_sb = cpool.tile([19, NSHARD], dt.bfloat16, tag="lhs")
        nc.sync.dma_start(out=lhs_sb[:], in_=lhsT)
        nodex_sb = cpool.tile([128, NTILES * 3], dt.float32, tag="nodex")
        nc.sync.dma_start(out=nodex_sb[:], in_=nodex)
        invb_sb = cpool.tile([128, POOL], dt.float32, tag="invb")
        nc.sync.dma_start(out=invb_sb[:], in_=invbase)
        eye_sb = cpool.tile([128, 128], dt.float32, tag="eye")
        nc.sync.dma_start(out=eye_sb[:], in_=eye)
        wp_sb = cpool.tile([128, 512], dt.float32, tag="wp")
        nc.sync.dma_start(out=wp_sb[:], in_=wp)
        wl1_sb = cpool.tile([128, 512], dt.float32, tag="wl1")
        nc.sync.dma_start(out=wl1_sb[:], in_=wl1)
        wl2_sb = cpool.tile([128, 512], dt.float32, tag="wl2")
        nc.sync.dma_start(out=wl2_sb[:], in_=wl2)
        bproj_sb = cpool.tile([128, 2], dt.float32, tag="bproj")
        nc.sync.dma_start(out=bproj_sb[:], in_=bproj)
        bl1_sb = cpool.tile([128, 2], dt.float32, tag="bl1")
        nc.sync.dma_start(out=bl1_sb[:], in_=bl1)
        bcomb_sb = cpool.tile([128, 2], dt.float32, tag="bcomb")
        nc.sync.dma_start(out=bcomb_sb[:], in_=bcomb)
        wt1_sb = cpool.tile([EMBED, EMBED], dt.float32, tag="wt1")
        nc.sync.dma_start(out=wt1_sb[:], in_=wt1)
        wt2_sb = cpool.tile([EMBED, C], dt.float32, tag="wt2")
        nc.sync.dma_start(out=wt2_sb[:], in_=wt2)
        bt1_sb = cpool.tile([EMBED, 1], dt.float32, tag="bt1")
        nc.sync.dma_start(out=bt1_sb[:], in_=bt1)
        fr_sb = cpool.tile([EMBED, 1], dt.float32, tag="fr")
        nc.sync.dma_start(out=fr_sb[:], in_=freqs)
        sh_sb = cpool.tile([EMBED, 1], dt.float32, tag="sh")
        nc.sync.dma_start(out=sh_sb[:], in_=shifts)
        t_sb = cpool.tile([EMBED, 1], dt.float32, tag="t1x1")
        nc.sync.dma_start(out=t_sb[:], in_=tval)

        # ---- t branch -> fincol [128, 2]
        e = cpool.tile([EMBED, 1], dt.float32, tag="e")
        nc.vector.tensor_mul(e[:], t_sb[:], fr_sb[:])
        nc.vector.tensor_add(e[:], e[:], sh_sb[:])
        ki = cpool.tile([EMBED, 1], dt.int32, tag="ki")
        kf = cpool.tile([EMBED, 1], dt.float32, tag="kf")
        nc.vector.tensor_scalar(kf[:], e[:], float(1.0 / (2 * PI)), None, op0=OP.mult)
        nc.vector.tensor_copy(out=ki[:], in_=kf[:])
        nc.vector.tensor_copy(out=kf[:], in_=ki[:])
        nc.vector.tensor_scalar(kf[:], kf[:], float(2 * PI), None, op0=OP.mult)
        nc.vector.tensor_sub(e[:], e[:], kf[:])
        gt = cpool.tile([EMBED, 1], dt.float32, tag="gt")
        nc.vector.tensor_scalar(gt[:], e[:], float(PI), None, op0=OP.is_gt)
        nc.vector.tensor_scalar(gt[:], gt[:], float(2 * PI), None, op0=OP.mult)
        nc.vector.tensor_sub(e[:], e[:], gt[:])
        emb_sb = cpool.tile([EMBED, 1], dt.float32, tag="emb")
        nc.scalar.activation(emb_sb[:], e[:], AF.Sin)
        ps_t1 = ppool.tile([EMBED, 1], dt.float32, tag="mm")
        nc.tensor.matmul(ps_t1[:], lhsT=wt1_sb[:], rhs=emb_sb[:], start=True, stop=True)
        h96 = cpool.tile([EMBED, 1], dt.float32, tag="h96")
        nc.scalar.activation(h96[:], ps_t1[:], AF.Identity, bias=bt1_sb[:, 0:1])
        h96b = cpool.tile([EMBED, 1], dt.float32, tag="h96b")
        nc.vector.tensor_scalar(h96b[:], h96[:], 0.1, None, op0=OP.mult)
        nc.vector.tensor_max(h96b[:], h96b[:], h96[:])
        fincol = cpool.tile([128, 2], dt.float32, tag="fincol")
        for d in range(2):
            ps_t2 = ppool.tile([128, 1], dt.float32, tag="mm")
            nc.tensor.matmul(ps_t2[:], lhsT=wt2_sb[:, d * 128:(d + 1) * 128],
                             rhs=h96b[:], start=True, stop=True)
            nc.scalar.activation(fincol[:, d:d + 1], ps_t2[:], AF.Identity,
                                 bias=bcomb_sb[:, d:d + 1])

        c32k = cpool.tile([128, NCAND], dt.float32, tag="c32k")
        nc.vector.memset(c32k[:], 32768.0)

        # ---- main loop, software-pipelined: A(t+1) emitted before B(t)
        def stageA(t):
            st = {}
            lt = lhs_sb[:, t * 128:(t + 1) * 128]
            pool_vals = wpool.tile([128, POOL], dt.float32, tag="pvals", name="pvals")
            pool_lidx = wpool.tile([128, POOL], dt.uint16, tag="plidx", name="plidx")
            for s in range(NSUP):
                rowb = wpool.tile([128, SUP], dt.float32, tag="rowb", name="rowb")
                for h in range(SUP // 1024):
                    ps_d = ppool.tile([128, 1024], dt.float32, tag="dist", name="psd")
                    for q in range(2):
                        c0 = s * SUP + h * 1024 + q * 512
                        nc.tensor.matmul(ps_d[:, q * 512:(q + 1) * 512], lhsT=lt,
                                         rhs=rhs_sb[:, c0:c0 + 512],
                                         start=True, stop=True)
                    nc.scalar.activation(rowb[:, h * 1024:(h + 1) * 1024], ps_d[:],
                                         AF.Identity)
                nc.vector.max(out=pool_vals[:, s * 8:(s + 1) * 8], in_=rowb[:])
                nc.vector.max_index(out=pool_lidx[:, s * 8:(s + 1) * 8],
                                    in_max=pool_vals[:, s * 8:(s + 1) * 8],
                                    in_values=rowb[:])
            lidxf = wpool.tile([128, POOL], dt.float32, tag="lidxf", name="lidxf")
            nc.vector.tensor_copy(out=lidxf[:], in_=pool_lidx[:])
            pinv = wpool.tile([128, POOL], dt.float32, tag="pinv", name="pinv")
            nc.vector.tensor_sub(pinv[:], invb_sb[:], lidxf[:])
            pv2 = wpool.tile([128, POOL], dt.float32, tag="pv2", name="pv2")
            pv3 = wpool.tile([128, POOL], dt.float32, tag="pv3", name="pv3")
            v8 = wpool.tile([128, 8], dt.float32, tag="v8", name="v8")
            nc.vector.max(out=v8[:], in_=pool_vals[:])
            nc.vector.match_replace(out=pv2[:], in_to_replace=v8[:],
                                    in_values=pool_vals[:], imm_value=-3e38)
            nc.vector.max(out=v8[:], in_=pv2[:])
            nc.vector.match_replace(out=pv3[:], in_to_replace=v8[:],
                                    in_values=pv2[:], imm_value=-3e38)
            maskp = wpool.tile([128, POOL], dt.float32, tag="maskp", name="maskp")
            nc.vector.tensor_tensor(out=maskp[:], in0=pv3[:], in1=pool_vals[:],
                                    op=OP.not_equal)
            nc.vector.tensor_mul(maskp[:], maskp[:], pinv[:])
            inv16 = wpool.tile([128, NCAND], dt.float32, tag="inv16", name="inv16")
            mv2 = wpool.tile([128, POOL], dt.float32, tag="mv2", name="mv2")
            nc.vector.max(out=inv16[:, 0:8], in_=maskp[:])
            nc.vector.match_replace(out=mv2[:], in_to_replace=inv16[:, 0:8],
                                    in_values=maskp[:], imm_value=0.0)
            nc.vector.max(out=inv16[:, 8:16], in_=mv2[:])
            g16f = wpool.tile([128, NCAND], dt.float32, tag="g16f", name="g16f")
            nc.vector.tensor_sub(g16f[:], c32k[:], inv16[:])
            g16u = wpool.tile([128, NCAND], dt.uint32, tag="g16u", name="g16u")
            nc.vector.tensor_copy(out=g16u[:], in_=g16f[:])
            gp = wpool.tile([128, NCAND, PACK], dt.float32, tag="gp", name="gp",
                            bufs=3)
            for k in range(NCAND):
                nc.gpsimd.indirect_dma_start(
                    out=gp[:, k, :], out_offset=None, in_=packed,
                    in_offset=bass.IndirectOffsetOnAxis(ap=g16u[:, k:k + 1], axis=0))
            st['gp'] = gp
            return st

        def stageB(t, st):
            gp = st['gp']
            xs = [nodex_sb[:, t * 3 + k: t * 3 + k + 1] for k in range(3)]
            dcol = [wpool.tile([128, NCAND], dt.float32, tag=f"d{k}",
                               name=f"dcol{k}") for k in range(3)]
            for k in range(3):
                nc.vector.tensor_scalar(dcol[k][:], gp[:, :, k + 1], xs[k],
                                        None, op0=OP.subtract)
            acc = wpool.tile([128, NCAND], dt.float32, tag="acc", name="acc")
            nc.vector.tensor_mul(acc[:], dcol[0][:], dcol[0][:])
            tt = [wpool.tile([128, NCAND], dt.float32, tag=f"t{i}",
                             name=f"tt{i}") for i in range(6)]
            for k in (1, 2):
                d = dcol[k]
                T0, T1, T2, T3, T4, T5 = tt
                nc.vector.tensor_scalar(T0[:], d[:], 4097.0, None, op0=OP.mult)
                nc.vector.tensor_sub(T1[:], T0[:], d[:])
                nc.vector.tensor_sub(T0[:], T0[:], T1[:])
                nc.vector.tensor_sub(T1[:], d[:], T0[:])
                nc.vector.tensor_mul(T2[:], d[:], d[:])
                nc.vector.tensor_mul(T3[:], T0[:], T0[:])
                nc.vector.tensor_sub(T3[:], T3[:], T2[:])
                nc.vector.tensor_add(T4[:], T1[:], T1[:])
                nc.vector.tensor_mul(T4[:], T0[:], T4[:])
                nc.vector.tensor_add(T3[:], T3[:], T4[:])
                nc.vector.tensor_mul(T4[:], T1[:], T1[:])
                nc.vector.tensor_add(T3[:], T3[:], T4[:])
                nc.vector.tensor_add(T4[:], T2[:], acc[:])
                nc.vector.tensor_sub(T5[:], T4[:], T2[:])
                nc.vector.tensor_sub(T0[:], T4[:], T5[:])
                nc.vector.tensor_sub(T0[:], T2[:], T0[:])
                nc.vector.tensor_sub(T1[:], acc[:], T5[:])
                nc.vector.tensor_add(T0[:], T0[:], T1[:])
                nc.vector.tensor_add(T0[:], T0[:], T3[:])
                nc.vector.tensor_add(acc[:], T4[:], T0[:])
            nd2 = wpool.tile([128, NCAND], dt.float32, tag="nd2", name="nd2")
            nc.vector.tensor_scalar(nd2[:], acc[:], -1.0, None, op0=OP.mult)
            v8f = wpool.tile([128, 8], dt.float32, tag="v8f", name="v8f")
            nc.vector.max(out=v8f[:], in_=nd2[:])
            refb = wpool.tile([128, NCAND], dt.float32, tag="refb", name="refb")
            nc.vector.match_replace(out=refb[:], in_to_replace=v8f[:],
                                    in_values=nd2[:], imm_value=3e38)
            mask2 = wpool.tile([128, NCAND], dt.float32, tag="mask2", name="mask2")
            nc.vector.tensor_tensor(out=mask2[:], in0=refb[:], in1=nd2[:],
                                    op=OP.not_equal)
            dist = wpool.tile([128, NCAND], dt.float32, tag="dist16", name="dist16")
            nc.scalar.activation(dist[:], acc[:], AF.Sqrt)
            nc.vector.tensor_scalar_max(dist[:], dist[:], 1e-6)
            wr = wpool.tile([128, NCAND], dt.float32, tag="wr", name="wr")
            nc.vector.reciprocal(out=wr[:], in_=dist[:])
            nc.vector.tensor_mul(wr[:], wr[:], mask2[:])
            wsum = wpool.tile([128, 1], dt.float32, tag="wsum", name="wsum")
            nc.vector.tensor_reduce(out=wsum[:], in_=wr[:],
                                    axis=mybir.AxisListType.X, op=OP.add)
            wsr = wpool.tile([128, 1], dt.float32, tag="wsr", name="wsr")
            nc.vector.reciprocal(out=wsr[:], in_=wsum[:])
            wn = wpool.tile([128, NCAND], dt.float32, tag="wn", name="wn")
            nc.vector.tensor_scalar(wn[:], wr[:], wsr[:, 0:1], None, op0=OP.mult)

            ps_fT = ppool.tile([128, C], dt.float32, tag="tr", name="psft")
            for k in range(NCAND):
                sc = wpool.tile([128, C], dt.float32, tag="sc", name="sc", bufs=3)
                nc.scalar.activation(sc[:], gp[:, k, 4:4 + C], AF.Identity,
                                     scale=wn[:, k:k + 1])
                for half_i in range(2):
                    nc.tensor.matmul(
                        ps_fT[:, half_i * 128:(half_i + 1) * 128],
                        lhsT=sc[:, half_i * 128:(half_i + 1) * 128],
                        rhs=eye_sb[:], is_transpose=True,
                        start=(k == 0 and half_i == 0),
                        stop=(k == NCAND - 1 and half_i == 1))
            fT = wpool.tile([128, C], dt.float32, tag="fT", name="fT")
            nc.scalar.activation(fT[:], ps_fT[:], AF.Identity)

            def dense(src_sb, wpack, bias_sb, leaky, outtag):
                o = wpool.tile([128, C], dt.float32, tag=outtag, name=outtag)
                for d in range(2):
                    ps = ppool.tile([128, 128], dt.float32, tag="mm", name="psmm")
                    for ct in range(2):
                        nc.tensor.matmul(
                            ps[:], lhsT=wpack[:, (ct * 2 + d) * 128:(ct * 2 + d + 1) * 128],
                            rhs=src_sb[:, ct * 128:(ct + 1) * 128],
                            start=(ct == 0), stop=(ct == 1))
                    nc.scalar.activation(o[:, d * 128:(d + 1) * 128], ps[:],
                                         AF.Identity, bias=bias_sb[:, d:d + 1])
                if leaky:
                    tmp = wpool.tile([128, C], dt.float32, tag=outtag + "lk",
                                     name=outtag + "lk")
                    nc.vector.tensor_scalar(tmp[:], o[:], 0.1, None, op0=OP.mult)
                    nc.vector.tensor_max(o[:], o[:], tmp[:])
                return o

            mT = dense(fT, wp_sb, bproj_sb, False, "mT")
            h1T = dense(mT, wl1_sb, bl1_sb, True, "h1T")
            pT = dense(h1T, wl2_sb, fincol, False, "pT")
            osb = wpool.tile([128, C], dt.float32, tag="osb", name="osb")
            for dth in range(2):
                ps_tr = ppool.tile([128, 128], dt.float32, tag="mm", name="pstr")
                nc.tensor.matmul(ps_tr[:], lhsT=pT[:, dth * 128:(dth + 1) * 128],
                                 rhs=eye_sb[:], is_transpose=True,
                                 start=True, stop=True)
                nc.scalar.activation(osb[:, dth * 128:(dth + 1) * 128], ps_tr[:],
                                     AF.Identity)
            nc.sync.dma_start(out=out[t * 128:(t + 1) * 128, :], in_=osb[:])

        pending = None
        for t in range(NTILES + 1):
            if t < NTILES:
                st = stageA(t)
            if pending is not None:
                stageB(t - 1, pending)
            pending = st if t < NTILES else None

    nc.compile()
    _CACHE['nc'] = nc
    return nc


# ---------------------------------------------------------------- host entry
def kernel(node_coords, cond_coords, cond_feats, t,
           W_proj, b_proj, W_l1, b_l1, W_l2, b_l2, W_t1, b_t1, W_t2, b_t2):
    node_coords = np.asarray(node_coords)
    cond_coords = np.asarray(cond_coords)
    cond_feats = np.asarray(cond_feats, dtype=np.float32)
    mc = np.float32(node_coords.astype(np.float32).max())
    part_c = _transform(cond_coords, 1.0, 0.01, mc)
    packed = np.ascontiguousarray(np.concatenate([part_c, cond_feats], 1))
    LHS, RHS = _split_rows(node_coords, cond_coords)
    full_c = _transform(node_coords, 16.0, 0.05, mc)

    invbase = np.tile((32768.0 - (np.arange(POOL) // 8) * SUP).astype(f32)[None, :],
                      (128, 1))
    eye = np.eye(128, dtype=f32)
    freqs1 = np.exp(np.arange(HALF, dtype=np.float32) *
                    f32(-math.log(10000.0) / (HALF - 1))).astype(f32)
    freqs = np.concatenate([freqs1, freqs1])
    shifts = np.concatenate([np.zeros(HALF, f32), np.full(HALF, PI / 2, f32)])

    nc = _build_program()
    in_maps = []
    for i in range(NCORES):
        sl = slice(i * NSHARD, (i + 1) * NSHARD)
        nodex = np.ascontiguousarray(
            full_c[sl, 1:4].reshape(NTILES, 128, 3).transpose(1, 0, 2)
            .reshape(128, NTILES * 3))
        in_maps.append({
            'lhsT': np.ascontiguousarray(LHS[:, sl]),
            'rhsT': RHS,
            'nodex': nodex,
            'packed': packed,
            'invbase': invbase,
            'eye': eye,
            'wp': _pack_w(np.asarray(W_proj, dtype=f32)),
            'wl1': _pack_w(np.asarray(W_l1, dtype=f32)),
            'wl2': _pack_w(np.asarray(W_l2, dtype=f32)),
            'bproj': np.asarray(b_proj, f32).reshape(2, 128).T.copy(),
            'bl1': np.asarray(b_l1, f32).reshape(2, 128).T.copy(),
            'bcomb': (np.asarray(b_l2, f32) + np.asarray(b_t2, f32)).reshape(2, 128).T.copy(),
            'wt1': np.ascontiguousarray(np.asarray(W_t1, f32).T),
            'wt2': np.ascontiguousarray(np.asarray(W_t2, f32).T),
            'bt1': np.asarray(b_t1, f32).reshape(EMBED, 1).copy(),
            'freqs': freqs.reshape(EMBED, 1).copy(),
            'shifts': shifts.reshape(EMBED, 1).copy(),
            'tval': np.full((EMBED, 1), np.asarray(t, f32).reshape(()), f32),
        })
    res = bass_utils.run_bass_kernel_spmd(nc, in_maps, core_ids=list(range(NCORES)))
    _CACHE['last_result'] = res
    outs = [res.results[i]['out'] for i in range(NCORES)]
    return np.concatenate(outs, 0)


# revision 45
# speedup vs baseline: 1.1344x; 1.1039x over previous
"""Trainium2 Bass kernel for AttentiveMinkUNetDiff KNN+MLP block (v2).

Self-contained: hardcodes shapes N=16384, M=32768, K=8, C=256, 8 cores.
Sharding: nodes across 8 cores; cond set replicated.

Per core (2048 nodes, 16 tiles of 128):
  1. PE: exact bf16-split integer matmul (K=19 rows) producing a
     per-node-ranking-equivalent of -40000*d^2 for all 32768 cond points.
  2. ACT copies PSUM->SBUF row buffers; DVE max/max_index per 8192-wide
     super -> 32-candidate pool with within-super indices.
  3. Top-16 of pool by value (match_replace rounds), then re-sorted by
     ascending cond index (jax.lax.top_k tie order) via masked-max trick.
  4. One packed indirect-DMA gather per candidate ([coords|feats] rows);
     exact d^2 recomputed bit-exactly vs XLA's fused fma chain (Dekker).
  5. Final 8 by exact value; inverse-distance weights; weighted mean of
     feats via ACT scaling + PE transpose-accumulate (weights sum to 1 so
     the mean commutes with W_proj); 3-layer MLP in transposed space;
     timestep-embedding branch folded into the final bias.
"""
import math
import numpy as np
import ml_dtypes

import concourse.bass as bass
import concourse.mybir as mybir
from concourse.tile import TileContext
from concourse import bass_utils
from concourse import bacc

bf16 = ml_dtypes.bfloat16
f32 = np.float32
AF = mybir.ActivationFunctionType
OP = mybir.AluOpType

N, M, K = 16384, 32768, 8
C = 256
PACK = C + 4                  # packed row: [part_c(4) | feats(256)]
EMBED, HALF = 96, 48
NCORES = 8
NSHARD = N // NCORES          # 2048
NTILES = NSHARD // 128        # 16
SUP = 4096                    # super-chunk width scanned from SBUF
NSUP = M // SUP               # 8
POOL = NSUP * 8               # 64
NCAND = 16
PI = float(np.pi)


# ---------------------------------------------------------------- host prep
def _split_rows(nodes, conds):
    """Build the 19 bf16-exact contraction rows. Validated vs reference."""
    a = nodes[:, 1:4].astype(np.int64)
    b = conds[:, 1:4].astype(np.int64)
    ah, al = a >> 5, a & 31
    bh, bl = b >> 5, b & 31
    lhs, rhs = [], []
    for k in range(3):
        lhs += [1280.0 * ah[:, k], 1280.0 * ah[:, k], 40.0 * al[:, k], 40.0 * al[:, k]]
        rhs += [32.0 * bh[:, k], 1.0 * bl[:, k], 32.0 * bh[:, k], 1.0 * bl[:, k]]
    B_total = (4 * b * b - 316 * b).sum(1) + 32768
    s2, s1, s0 = B_total >> 16, (B_total >> 8) & 255, B_total & 255
    nones = -np.ones(a.shape[0])
    lhs += [nones, nones, nones]
    rhs += [s2 * 65536.0, s1 * 256.0, s0 * 1.0]
    C_i = ((10 * a + 79) ** 2).sum(1) - 32768
    c3 = np.floor(C_i / 2 ** 21).astype(np.int64)
    r = C_i - c3 * 2 ** 21
    c2, c1, c0 = r >> 13, (r >> 5) & 255, r & 31
    mones = np.ones(b.shape[0])
    lhs += [-c3 * 2097152.0, -c2 * 8192.0, -c1 * 32.0, -c0 * 1.0]
    rhs += [mones, mones, mones, mones]
    LHS = np.stack(lhs).astype(f32)   # [19, N]
    RHS = np.stack(rhs).astype(f32)   # [19, M]
    return LHS.astype(bf16), RHS.astype(bf16)


def _transform(coords, stride, voxel, mc):
    c = coords.astype(np.float32)
    batch = (c[:, :1] * f32(mc * f32(2.0))).astype(f32)
    xyz = ((c[:, 1:] + f32(stride / 2.0)).astype(f32) * f32(voxel)).astype(f32)
    return np.concatenate([batch, xyz], 1).astype(f32)


def _pack_w(w):
    """W [dout, din] -> lhsT pack [128, 4*128]: col block (ct*2+dt)."""
    wt = np.ascontiguousarray(w.T.astype(f32))          # [din, dout]
    p = wt.reshape(2, 128, 2, 128)                      # [ct, c, dt, d]
    p = p.transpose(1, 0, 2, 3).reshape(128, 512)
    return np.ascontiguousarray(p)


_CACHE = {}


def _build_program():
    if 'nc' in _CACHE:
        return _CACHE['nc']
    nc = bacc.Bacc("TRN2", target_bir_lowering=False, debug=False,
                   num_devices=NCORES)
    dt = mybir.dt

    def din(name, shape, dtype):
        return nc.dram_tensor(name, shape, dtype, kind="ExternalInput").ap()

    lhsT = din('lhsT', [19, NSHARD], dt.bfloat16)
    rhs = din('rhsT', [19, M], dt.bfloat16)
    nodex = din('nodex', [128, NTILES * 3], dt.float32)
    packed = din('packed', [M, PACK], dt.float32)
    invbase = din('invbase', [128, POOL], dt.float32)
    eye = din('eye', [128, 128], dt.float32)
    wp = din('wp', [128, 512], dt.float32)
    wl1 = din('wl1', [128, 512], dt.float32)
    wl2 = din('wl2', [128, 512], dt.float32)
    bproj = din('bproj', [128, 2], dt.float32)
    bl1 = din('bl1', [128, 2], dt.float32)
    bcomb = din('bcomb', [128, 2], dt.float32)
    wt1 = din('wt1', [EMBED, EMBED], dt.float32)
    wt2 = din('wt2', [EMBED, C], dt.float32)
    bt1 = din('bt1', [EMBED, 1], dt.float32)
    freqs = din('freqs', [EMBED, 1], dt.float32)
    shifts = din('shifts', [EMBED, 1], dt.float32)
    tval = din('tval', [EMBED, 1], dt.float32)
    out = nc.dram_tensor('out', [NSHARD, C], dt.float32, kind="ExternalOutput").ap()

    with TileContext(nc) as tc, \
            tc.tile_pool(name="const", bufs=1) as cpool, \
            tc.tile_pool(name="work", bufs=2) as wpool, \
            tc.tile_pool(name="psum", bufs=2, space="PSUM") as ppool:

        # ---- constants to SBUF
        rhs_sb = cpool.tile([19, M], dt.bfloat16, tag="rhs")
        for j in range(8):
            nc.sync.dma_start(out=rhs_sb[:, j * (M // 8):(j + 1) * (M // 8)],
                              in_=rhs[:, j * (M // 8):(j + 1) * (M // 8)])
        lhs_sb = cpool.tile([19, NSHARD], dt.bfloat16, tag="lhs")
        nc.sync.dma_start(out=lhs_sb[:], in_=lhsT)
        nodex_sb = cpool.tile([128, NTILES * 3], dt.float32, tag="nodex")
        nc.sync.dma_start(out=nodex_sb[:], in_=nodex)
        invb_sb = cpool.tile([128, POOL], dt.float32, tag="invb")
        nc.sync.dma_start(out=invb_sb[:], in_=invbase)
        eye_sb = cpool.tile([128, 128], dt.float32, tag="eye")
        nc.sync.dma_start(out=eye_sb[:], in_=eye)
        wp_sb = cpool.tile([128, 512], dt.float32, tag="wp")
        nc.sync.dma_start(out=wp_sb[:], in_=wp)
        wl1_sb = cpool.tile([128, 512], dt.float32, tag="wl1")
        nc.sync.dma_start(out=wl1_sb[:], in_=wl1)
        wl2_sb = cpool.tile([128, 512], dt.float32, tag="wl2")
        nc.sync.dma_start(out=wl2_sb[:], in_=wl2)
        bproj_sb = cpool.tile([128, 2], dt.float32, tag="bproj")
        nc.sync.dma_start(out=bproj_sb[:], in_=bproj)
        bl1_sb = cpool.tile([128, 2], dt.float32, tag="bl1")
        nc.sync.dma_start(out=bl1_sb[:], in_=bl1)
        bcomb_sb = cpool.tile([128, 2], dt.float32, tag="bcomb")
        nc.sync.dma_start(out=bcomb_sb[:], in_=bcomb)
        wt1_sb = cpool.tile([EMBED, EMBED], dt.float32, tag="wt1")
        nc.sync.dma_start(out=wt1_sb[:], in_=wt1)
        wt2_sb = cpool.tile([EMBED, C], dt.float32, tag="wt2")
        nc.sync.dma_start(out=wt2_sb[:], in_=wt2)
        bt1_sb = cpool.tile([EMBED, 1], dt.float32, tag="bt1")
        nc.sync.dma_start(out=bt1_sb[:], in_=bt1)
        fr_sb = cpool.tile([EMBED, 1], dt.float32, tag="fr")
        nc.sync.dma_start(out=fr_sb[:], in_=freqs)
        sh_sb = cpool.tile([EMBED, 1], dt.float32, tag="sh")
        nc.sync.dma_start(out=sh_sb[:], in_=shifts)
        t_sb = cpool.tile([EMBED, 1], dt.float32, tag="t1x1")
        nc.sync.dma_start(out=t_sb[:], in_=tval)

        # ---- t branch -> fincol [128, 2]
        e = cpool.tile([EMBED, 1], dt.float32, tag="e")
        nc.vector.tensor_mul(e[:], t_sb[:], fr_sb[:])
        nc.vector.tensor_add(e[:], e[:], sh_sb[:])
        ki = cpool.tile([EMBED, 1], dt.int32, tag="ki")
        kf = cpool.tile([EMBED, 1], dt.float32, tag="kf")
        nc.vector.tensor_scalar(kf[:], e[:], float(1.0 / (2 * PI)), None, op0=OP.mult)
        nc.vector.tensor_copy(out=ki[:], in_=kf[:])
        nc.vector.tensor_copy(out=kf[:], in_=ki[:])
        nc.vector.tensor_scalar(kf[:], kf[:], float(2 * PI), None, op0=OP.mult)
        nc.vector.tensor_sub(e[:], e[:], kf[:])
        gt = cpool.tile([EMBED, 1], dt.float32, tag="gt")
        nc.vector.tensor_scalar(gt[:], e[:], float(PI), None, op0=OP.is_gt)
        nc.vector.tensor_scalar(gt[:], gt[:], float(2 * PI), None, op0=OP.mult)
        nc.vector.tensor_sub(e[:], e[:], gt[:])
        emb_sb = cpool.tile([EMBED, 1], dt.float32, tag="emb")
        nc.scalar.activation(emb_sb[:], e[:], AF.Sin)
        ps_t1 = ppool.tile([EMBED, 1], dt.float32, tag="mm")
        nc.tensor.matmul(ps_t1[:], lhsT=wt1_sb[:], rhs=emb_sb[:], start=True, stop=True)
        h96 = cpool.tile([EMBED, 1], dt.float32, tag="h96")
        nc.scalar.activation(h96[:], ps_t1[:], AF.Identity, bias=bt1_sb[:, 0:1])
        h96b = cpool.tile([EMBED, 1], dt.float32, tag="h96b")
        nc.vector.tensor_scalar(h96b[:], h96[:], 0.1, None, op0=OP.mult)
        nc.vector.tensor_max(h96b[:], h96b[:], h96[:])
        fincol = cpool.tile([128, 2], dt.float32, tag="fincol")
        for d in range(2):
            ps_t2 = ppool.tile([128, 1], dt.float32, tag="mm")
            nc.tensor.matmul(ps_t2[:], lhsT=wt2_sb[:, d * 128:(d + 1) * 128],
                             rhs=h96b[:], start=True, stop=True)
            nc.scalar.activation(fincol[:, d:d + 1], ps_t2[:], AF.Identity,
                                 bias=bcomb_sb[:, d:d + 1])

        c32k = cpool.tile([128, NCAND], dt.float32, tag="c32k")
        nc.vector.memset(c32k[:], 32768.0)

        # ---- main loop, software-pipelined: A(t+1) emitted before B(t)
        def stageA(t):
            st = {}
            lt = lhs_sb[:, t * 128:(t + 1) * 128]
            pool_vals = wpool.tile([128, POOL], dt.float32, tag="pvals", name="pvals", bufs=3)
            pool_lidx = wpool.tile([128, POOL], dt.uint16, tag="plidx", name="plidx", bufs=3)
            for s in range(NSUP):
                rowb = wpool.tile([128, SUP], dt.float32, tag="rowb", name="rowb", bufs=3)
                for h in range(SUP // 1024):
                    ps_d = ppool.tile([128, 1024], dt.float32, tag="dist", name="psd")
                    for q in range(2):
                        c0 = s * SUP + h * 1024 + q * 512
                        nc.tensor.matmul(ps_d[:, q * 512:(q + 1) * 512], lhsT=lt,
                                         rhs=rhs_sb[:, c0:c0 + 512],
                                         start=True, stop=True)
                    nc.scalar.activation(rowb[:, h * 1024:(h + 1) * 1024], ps_d[:],
                                         AF.Identity)
                nc.vector.max(out=pool_vals[:, s * 8:(s + 1) * 8], in_=rowb[:])
                nc.vector.max_index(out=pool_lidx[:, s * 8:(s + 1) * 8],
                                    in_max=pool_vals[:, s * 8:(s + 1) * 8],
                                    in_values=rowb[:])
            lidxf = wpool.tile([128, POOL], dt.float32, tag="lidxf", name="lidxf")
            nc.vector.tensor_copy(out=lidxf[:], in_=pool_lidx[:])
            pinv = wpool.tile([128, POOL], dt.float32, tag="pinv", name="pinv")
            nc.vector.tensor_sub(pinv[:], invb_sb[:], lidxf[:])
            pv2 = wpool.tile([128, POOL], dt.float32, tag="pv2", name="pv2")
            pv3 = wpool.tile([128, POOL], dt.float32, tag="pv3", name="pv3")
            v8 = wpool.tile([128, 8], dt.float32, tag="v8", name="v8")
            nc.vector.max(out=v8[:], in_=pool_vals[:])
            nc.vector.match_replace(out=pv2[:], in_to_replace=v8[:],
                                    in_values=pool_vals[:], imm_value=-3e38)
            nc.vector.max(out=v8[:], in_=pv2[:])
            nc.vector.match_replace(out=pv3[:], in_to_replace=v8[:],
                                    in_values=pv2[:], imm_value=-3e38)
            maskp = wpool.tile([128, POOL], dt.float32, tag="maskp", name="maskp")
            nc.vector.tensor_tensor(out=maskp[:], in0=pv3[:], in1=pool_vals[:],
                                    op=OP.not_equal)
            nc.vector.tensor_mul(maskp[:], maskp[:], pinv[:])
            inv16 = wpool.tile([128, NCAND], dt.float32, tag="inv16", name="inv16")
            mv2 = wpool.tile([128, POOL], dt.float32, tag="mv2", name="mv2")
            nc.vector.max(out=inv16[:, 0:8], in_=maskp[:])
            nc.vector.match_replace(out=mv2[:], in_to_replace=inv16[:, 0:8],
                                    in_values=maskp[:], imm_value=0.0)
            nc.vector.max(out=inv16[:, 8:16], in_=mv2[:])
            g16f = wpool.tile([128, NCAND], dt.float32, tag="g16f", name="g16f")
            nc.vector.tensor_sub(g16f[:], c32k[:], inv16[:])
            g16u = wpool.tile([128, NCAND], dt.uint32, tag="g16u", name="g16u")
            nc.vector.tensor_copy(out=g16u[:], in_=g16f[:])
            gp = wpool.tile([128, NCAND, PACK], dt.float32, tag="gp", name="gp",
                            bufs=3)
            for k in range(NCAND):
                nc.gpsimd.indirect_dma_start(
                    out=gp[:, k, :], out_offset=None, in_=packed,
                    in_offset=bass.IndirectOffsetOnAxis(ap=g16u[:, k:k + 1], axis=0))
            st['gp'] = gp
            return st

        def stageB(t, st):
            gp = st['gp']
            xs = [nodex_sb[:, t * 3 + k: t * 3 + k + 1] for k in range(3)]
            dcol = [wpool.tile([128, NCAND], dt.float32, tag=f"d{k}",
                               name=f"dcol{k}") for k in range(3)]
            for k in range(3):
                nc.vector.tensor_scalar(dcol[k][:], gp[:, :, k + 1], xs[k],
                                        None, op0=OP.subtract)
            acc = wpool.tile([128, NCAND], dt.float32, tag="acc", name="acc")
            nc.vector.tensor_mul(acc[:], dcol[0][:], dcol[0][:])
            tt = [wpool.tile([128, NCAND], dt.float32, tag=f"t{i}",
                             name=f"tt{i}") for i in range(6)]
            for k in (1, 2):
                d = dcol[k]
                T0, T1, T2, T3, T4, T5 = tt
                nc.vector.tensor_scalar(T0[:], d[:], 4097.0, None, op0=OP.mult)
                nc.vector.tensor_sub(T1[:], T0[:], d[:])
                nc.vector.tensor_sub(T0[:], T0[:], T1[:])
                nc.vector.tensor_sub(T1[:], d[:], T0[:])
                nc.vector.tensor_mul(T2[:], d[:], d[:])
                nc.vector.tensor_mul(T3[:], T0[:], T0[:])
                nc.vector.tensor_sub(T3[:], T3[:], T2[:])
                nc.vector.tensor_add(T4[:], T1[:], T1[:])
                nc.vector.tensor_mul(T4[:], T0[:], T4[:])
                nc.vector.tensor_add(T3[:], T3[:], T4[:])
                nc.vector.tensor_mul(T4[:], T1[:], T1[:])
                nc.vector.tensor_add(T3[:], T3[:], T4[:])
                nc.vector.tensor_add(T4[:], T2[:], acc[:])
                nc.vector.tensor_sub(T5[:], T4[:], T2[:])
                nc.vector.tensor_sub(T0[:], T4[:], T5[:])
                nc.vector.tensor_sub(T0[:], T2[:], T0[:])
                nc.vector.tensor_sub(T1[:], acc[:], T5[:])
                nc.vector.tensor_add(T0[:], T0[:], T1[:])
                nc.vector.tensor_add(T0[:], T0[:], T3[:])
                nc.vector.tensor_add(acc[:], T4[:], T0[:])
            nd2 = wpool.tile([128, NCAND], dt.float32, tag="nd2", name="nd2")
            nc.vector.tensor_scalar(nd2[:], acc[:], -1.0, None, op0=OP.mult)
            v8f = wpool.tile([128, 8], dt.float32, tag="v8f", name="v8f")
            nc.vector.max(out=v8f[:], in_=nd2[:])
            refb = wpool.tile([128, NCAND], dt.float32, tag="refb", name="refb")
            nc.vector.match_replace(out=refb[:], in_to_replace=v8f[:],
                                    in_values=nd2[:], imm_value=3e38)
            mask2 = wpool.tile([128, NCAND], dt.float32, tag="mask2", name="mask2")
            nc.vector.tensor_tensor(out=mask2[:], in0=refb[:], in1=nd2[:],
                                    op=OP.not_equal)
            dist = wpool.tile([128, NCAND], dt.float32, tag="dist16", name="dist16")
            nc.scalar.activation(dist[:], acc[:], AF.Sqrt)
            nc.vector.tensor_scalar_max(dist[:], dist[:], 1e-6)
            wr = wpool.tile([128, NCAND], dt.float32, tag="wr", name="wr")
            nc.vector.reciprocal(out=wr[:], in_=dist[:])
            nc.vector.tensor_mul(wr[:], wr[:], mask2[:])
            wsum = wpool.tile([128, 1], dt.float32, tag="wsum", name="wsum")
            nc.vector.tensor_reduce(out=wsum[:], in_=wr[:],
                                    axis=mybir.AxisListType.X, op=OP.add)
            wsr = wpool.tile([128, 1], dt.float32, tag="wsr", name="wsr")
            nc.vector.reciprocal(out=wsr[:], in_=wsum[:])
            wn = wpool.tile([128, NCAND], dt.float32, tag="wn", name="wn")
            nc.vector.tensor_scalar(wn[:], wr[:], wsr[:, 0:1], None, op0=OP.mult)

            ps_fT = ppool.tile([128, C], dt.float32, tag="tr", name="psft")
            for k in range(NCAND):
                sc = wpool.tile([128, C], dt.float32, tag="sc", name="sc", bufs=4)
                nc.scalar.activation(sc[:], gp[:, k, 4:4 + C], AF.Identity,
                                     scale=wn[:, k:k + 1])
                for half_i in range(2):
                    nc.tensor.matmul(
                        ps_fT[:, half_i * 128:(half_i + 1) * 128],
                        lhsT=sc[:, half_i * 128:(half_i + 1) * 128],
                        rhs=eye_sb[:], is_transpose=True,
                        start=(k == 0 and half_i == 0),
                        stop=(k == NCAND - 1 and half_i == 1))
            fT = wpool.tile([128, C], dt.float32, tag="fT", name="fT")
            nc.scalar.activation(fT[:], ps_fT[:], AF.Identity)

            def dense(src_sb, wpack, bias_sb, leaky, outtag):
                o = wpool.tile([128, C], dt.float32, tag=outtag, name=outtag)
                for d in range(2):
                    ps = ppool.tile([128, 128], dt.float32, tag="mm", name="psmm")
                    for ct in range(2):
                        nc.tensor.matmul(
                            ps[:], lhsT=wpack[:, (ct * 2 + d) * 128:(ct * 2 + d + 1) * 128],
                            rhs=src_sb[:, ct * 128:(ct + 1) * 128],
                            start=(ct == 0), stop=(ct == 1))
                    nc.scalar.activation(o[:, d * 128:(d + 1) * 128], ps[:],
                                         AF.Identity, bias=bias_sb[:, d:d + 1])
                if leaky:
                    tmp = wpool.tile([128, C], dt.float32, tag=outtag + "lk",
                                     name=outtag + "lk")
                    nc.vector.tensor_scalar(tmp[:], o[:], 0.1, None, op0=OP.mult)
                    nc.vector.tensor_max(o[:], o[:], tmp[:])
                return o

            mT = dense(fT, wp_sb, bproj_sb, False, "mT")
            h1T = dense(mT, wl1_sb, bl1_sb, True, "h1T")
            pT = dense(h1T, wl2_sb, fincol, False, "pT")
            osb = wpool.tile([128, C], dt.float32, tag="osb", name="osb")
            for dth in range(2):
                ps_tr = ppool.tile([128, 128], dt.float32, tag="mm", name="pstr")
                nc.tensor.matmul(ps_tr[:], lhsT=pT[:, dth * 128:(dth + 1) * 128],
                                 rhs=eye_sb[:], is_transpose=True,
                                 start=True, stop=True)
                nc.scalar.activation(osb[:, dth * 128:(dth + 1) * 128], ps_tr[:],
                                     AF.Identity)
            nc.sync.dma_start(out=out[t * 128:(t + 1) * 128, :], in_=osb[:])

        pending = None
        for t in range(NTILES + 1):
            if t < NTILES:
                st = stageA(t)
            if pending is not None:
                stageB(t - 1, pending)
            pending = st if t < NTILES else None

    nc.compile()
    _CACHE['nc'] = nc
    return nc


# ---------------------------------------------------------------- host entry
def kernel(node_coords, cond_coords, cond_feats, t,
           W_proj, b_proj, W_l1, b_l1, W_l2, b_l2, W_t1, b_t1, W_t2, b_t2):
    node_coords = np.asarray(node_coords)
    cond_coords = np.asarray(cond_coords)
    cond_feats = np.asarray(cond_feats, dtype=np.float32)
    mc = np.float32(node_coords.astype(np.float32).max())
    part_c = _transform(cond_coords, 1.0, 0.01, mc)
    packed = np.ascontiguousarray(np.concatenate([part_c, cond_feats], 1))
    LHS, RHS = _split_rows(node_coords, cond_coords)
    full_c = _transform(node_coords, 16.0, 0.05, mc)

    invbase = np.tile((32768.0 - (np.arange(POOL) // 8) * SUP).astype(f32)[None, :],
                      (128, 1))
    eye = np.eye(128, dtype=f32)
    freqs1 = np.exp(np.arange(HALF, dtype=np.float32) *
                    f32(-math.log(10000.0) / (HALF - 1))).astype(f32)
    freqs = np.concatenate([freqs1, freqs1])
    shifts = np.concatenate([np.zeros(HALF, f32), np.full(HALF, PI / 2, f32)])

    nc = _build_program()
    in_maps = []
    for i in range(NCORES):
        sl = slice(i * NSHARD, (i + 1) * NSHARD)
        nodex = np.ascontiguousarray(
            full_c[sl, 1:4].reshape(NTILES, 128, 3).transpose(1, 0, 2)
            .reshape(128, NTILES * 3))
        in_maps.append({
            'lhsT': np.ascontiguousarray(LHS[:, sl]),
            'rhsT': RHS,
            'nodex': nodex,
            'packed': packed,
            'invbase': invbase,
            'eye': eye,
            'wp': _pack_w(np.asarray(W_proj, dtype=f32)),
            'wl1': _pack_w(np.asarray(W_l1, dtype=f32)),
            'wl2': _pack_w(np.asarray(W_l2, dtype=f32)),
            'bproj': np.asarray(b_proj, f32).reshape(2, 128).T.copy(),
            'bl1': np.asarray(b_l1, f32).reshape(2, 128).T.copy(),
            'bcomb': (np.asarray(b_l2, f32) + np.asarray(b_t2, f32)).reshape(2, 128).T.copy(),
            'wt1': np.ascontiguousarray(np.asarray(W_t1, f32).T),
            'wt2': np.ascontiguousarray(np.asarray(W_t2, f32).T),
            'bt1': np.asarray(b_t1, f32).reshape(EMBED, 1).copy(),
            'freqs': freqs.reshape(EMBED, 1).copy(),
            'shifts': shifts.reshape(EMBED, 1).copy(),
            'tval': np.full((EMBED, 1), np.asarray(t, f32).reshape(()), f32),
        })
    res = bass_utils.run_bass_kernel_spmd(nc, in_maps, core_ids=list(range(NCORES)))
    _CACHE['last_result'] = res
    outs = [res.results[i]['out'] for i in range(NCORES)]
    return np.concatenate(outs, 0)
